# revision 13
# baseline (speedup 1.0000x reference)
"""Trainium2 Bass kernel for the dense_cnn problem.

Math (per sample, C=256, H=W=56, HW=3136, G=2, K=128):
  t1 = p1*x
  t2 = t1[c,hw] @ p2[hw,k]                  (computed transposed: t2T[k,c])
  t3 = t1 @ x.T / sqrt(hw)                  (computed transposed: t3T[d,c])
  t4 = grouped dilated 3x1 conv of t2 (only middle kw tap contributes)
  t5 = w5 @ x
  t8 = grouped dilated 3x3 conv of x (9 shifted block-diag matmuls)
  t9 = max(t5, t8)
  out = (t4 - t3/sqrt(hw)) @ t9 / sqrt(c)

Distribution: pure data-parallel over batch, 4 samples per core x 8 cores.
Layouts: hw-contraction ops run on a transposed copy of x ([hw, c], padded to
3200 rows); spatial ops run on a width-padded copy ([c, 56 x 62]) so all conv
taps become column-shifted matmuls with correct zero boundaries.
All tensors are bf16 (PSUM accumulation stays f32): halves HBM traffic and
DVE element count vs f32, and bf16 matmuls run at full PE rate at any
free-dim size (f32 pays 4x, f32r pays 4x below free-dim 256).
"""

import numpy as np
import ml_dtypes

import concourse.bass as bass
import concourse.tile as tile
from concourse import bacc, mybir
from concourse.bass_utils import run_bass_kernel_spmd

N, C, H, W = 32, 256, 56, 56
HW = H * W              # 3136
PW = W + 6              # width padded by 3 each side: 62
HWP = H * PW            # 3472
NCORE = 8
SPC = N // NCORE        # samples per core: 4
NCHUNK = 25             # hw-contraction chunks of 128 (rows padded to 3200)
HWPAD = NCHUNK * 128    # 3200
NCH2 = 7                # phase-2 column chunks
CH_SP = HWP // NCH2     # 496 padded cols per chunk (8 image rows)
CH_OUT = CH_SP - 48     # 448 compact cols per chunk
ROWS_PER_CHUNK = 8
XM = 192                # zero margin around each padded half (> max |shift| 189)
HWPM = HWP + 2 * XM     # 3856

F32 = mybir.dt.float32
BF16 = mybir.dt.bfloat16
NPBF = ml_dtypes.bfloat16
MUL = mybir.AluOpType.mult
ADD = mybir.AluOpType.add

_PROGRAM_CACHE: dict = {}


def _build_program():
    nc = bacc.Bacc("TRN2", target_bir_lowering=False, debug=False,
                   num_devices=NCORE)

    d = {}
    def din(name, shape, dt=BF16):
        d[name] = nc.dram_tensor(name, list(shape), dt, kind="ExternalInput").ap()
    # constants are stored partition-major host-side so every DMA row is one
    # big contiguous descriptor (>=512B avoids the 2x small-transfer penalty)
    din("xpad", (SPC, 2, 128, HWPM))
    din("xt", (SPC, NCHUNK, 128, 256))
    din("p1t", (128, NCHUNK * 256))
    din("p2f", (128, NCHUNK * 128))
    din("w4t", (128, 6 * 128))
    din("w8t", (128, 18 * 128))
    din("w5t", (128, 4 * 128))
    din("ident", (128, 128))
    out_dram = nc.dram_tensor("out", [SPC, 2, 128, HW], BF16,
                              kind="ExternalOutput").ap()

    with tile.TileContext(nc) as tc:
        _emit(tc, nc, d, out_dram)
    nc.compile()
    return nc


def _emit(tc, nc, d, out_dram):
    from contextlib import ExitStack
    ctx = ExitStack()
    with ctx:
        const = ctx.enter_context(tc.tile_pool(name="const", bufs=1))
        xt_pool = ctx.enter_context(tc.tile_pool(name="xt", bufs=4))
        t1_pool = ctx.enter_context(tc.tile_pool(name="t1", bufs=4))
        xpad_pool = ctx.enter_context(tc.tile_pool(name="xpad", bufs=2))
        t9_pool = ctx.enter_context(tc.tile_pool(name="t9", bufs=2))
        sb_small = ctx.enter_context(tc.tile_pool(name="sbs", bufs=2))
        out_pool = ctx.enter_context(tc.tile_pool(name="outp", bufs=3))
        # PSUM (16KB/partition): acc 3x1KB + smps 2x1.75KB + t8 2x2KB +
        # t5 2x2KB = 14.5KB
        acc_ps = ctx.enter_context(tc.tile_pool(name="accps", bufs=1, space="PSUM"))
        small_ps = ctx.enter_context(tc.tile_pool(name="smps", bufs=2, space="PSUM"))
        sp_ps = ctx.enter_context(tc.tile_pool(name="spps", bufs=1, space="PSUM"))

        # ---- load constants -------------------------------------------------
        # spatial weights first: the first PE work (t8 of sample 0) needs only
        # w8t/w5t + xpad, so don't serialize it behind the big phase-1 consts.
        w8t_sb = const.tile([128, 18 * 128], BF16)
        nc.sync.dma_start(w8t_sb[:], d["w8t"])
        w5t_sb = const.tile([128, 4 * 128], BF16)
        nc.sync.dma_start(w5t_sb[:], d["w5t"])
        # sample 0's xpad in two pieces: the first covers chunks j=0..2 of
        # both halves, so the first t8 matmuls start ~3us earlier (tile deps
        # are range-based)
        XSPL = 2200
        xpad_t0 = xpad_pool.tile([128, 2 * HWPM], BF16, tag="xpad")
        for gg in range(2):
            nc.sync.dma_start(xpad_t0[:, gg * HWPM:gg * HWPM + XSPL],
                              d["xpad"][0, gg][:, :XSPL])
        for gg in range(2):
            nc.sync.dma_start(xpad_t0[:, gg * HWPM + XSPL:(gg + 1) * HWPM],
                              d["xpad"][0, gg][:, XSPL:])
        p1t_sb = const.tile([128, NCHUNK * 256], BF16)
        p2f_sb = const.tile([128, NCHUNK * 128], BF16)
        for gi in range(5):
            i0, i1 = gi * 5, gi * 5 + 5
            nc.sync.dma_start(p1t_sb[:, i0 * 256:i1 * 256],
                              d["p1t"][:, i0 * 256:i1 * 256])
            nc.sync.dma_start(p2f_sb[:, i0 * 128:i1 * 128],
                              d["p2f"][:, i0 * 128:i1 * 128])
        w4t_sb = const.tile([128, 6 * 128], BF16)
        nc.sync.dma_start(w4t_sb[:], d["w4t"])
        id_sb = const.tile([128, 128], BF16)
        nc.sync.dma_start(id_sb[:], d["ident"])
        # padded t2 staging ([128, 2 x 134], pad cols stay zero)
        t2p_sb = const.tile([128, 2 * 134], BF16)
        nc.gpsimd.memset(t2p_sb[:], 0.0)

        # w4t carries the final 1/sqrt(C) scale (folded host-side), so the
        # final matmul result needs no epilogue scaling.
        inv56_16 = float(1.0 / np.float32(np.sqrt(np.float32(HW)))
                         / np.float32(np.sqrt(np.float32(C))))

        # tap order for t8: (1,1) first (always full coverage -> start=True)
        taps = [(1, 1)] + [(kh, kw) for kh in range(3) for kw in range(3)
                           if (kh, kw) != (1, 1)]

        def emit_spatial(s, xpad_t):
            # t5/t8/t9 for every column chunk; needs only xpad + w5t/w8t.
            t9_sb = []
            for j in range(NCH2):
                c0, c1 = j * CH_SP, (j + 1) * CH_SP
                for g in range(2):
                    t8_ps = sp_ps.tile([128, CH_SP], F32, tag="t8", bufs=2)
                    for ti, (kh, kw) in enumerate(taps):
                        sh = 3 * PW * (kh - 1) + 3 * (kw - 1)
                        widx = (kh * 3 + kw) * 2 + g
                        ro = g * HWPM + XM + c0 + sh
                        nc.tensor.matmul(
                            t8_ps[:],
                            w8t_sb[:, widx * 128:(widx + 1) * 128],
                            xpad_t[:, ro:ro + CH_SP],
                            start=(ti == 0), stop=(ti == len(taps) - 1))
                    t5_ps = sp_ps.tile([128, CH_SP], F32, tag="t5", bufs=1)
                    for cc in range(2):
                        nc.tensor.matmul(
                            t5_ps[:],
                            w5t_sb[:, (g * 2 + cc) * 128:(g * 2 + cc + 1) * 128],
                            xpad_t[:, cc * HWPM + XM + c0:cc * HWPM + XM + c1],
                            start=(cc == 0), stop=(cc == 1))
                    # stage t8 to SBUF (walrus rejects TensorTensor with two
                    # PSUM operands), then max t5_ps against it directly with
                    # the 62 -> 56 column compaction.
                    t8_sb = out_pool.tile([128, CH_SP], BF16, tag="t8sb", bufs=2)
                    nc.scalar.copy(t8_sb[:], t8_ps[:])
                    t9_g = t9_pool.tile([128, CH_OUT], BF16, tag=f"t9c{j}{g}",
                                        bufs=2)
                    nc.vector.tensor_max(
                        t9_g[:].rearrange("p (r c) -> p r c", c=56),
                        t5_ps[:].rearrange("p (r c) -> p r c", c=62)[:, :, 3:59],
                        t8_sb[:].rearrange("p (r c) -> p r c", c=62)[:, :, 3:59])
                    t9_sb.append(t9_g)
            return t9_sb

        def emit_phase1(s):
            t2T_ps = acc_ps.tile([128, 256], F32, tag="t2T")
            t3T_ps = [acc_ps.tile([128, 256], F32, name=f"t3T{g}", tag=f"t3T{g}")
                      for g in range(2)]
            for i in range(NCHUNK):
                xt_t = xt_pool.tile([128, 256], BF16)
                nc.sync.dma_start(xt_t[:], d["xt"][s, i])
                t1_t = t1_pool.tile([128, 256], BF16)
                nc.vector.tensor_mul(t1_t[:], xt_t[:],
                                     p1t_sb[:, i * 256:(i + 1) * 256])
                fl = dict(start=(i == 0), stop=(i == NCHUNK - 1))
                nc.tensor.matmul(t2T_ps[:], p2f_sb[:, i * 128:(i + 1) * 128],
                                 t1_t[:], **fl)
                for g in range(2):
                    nc.tensor.matmul(t3T_ps[g][:],
                                     xt_t[:, g * 128:(g + 1) * 128],
                                     t1_t[:], **fl)
            return t2T_ps, t3T_ps

        def emit_chain(t2T_ps, t3T_ps):
            t2T_sb = sb_small.tile([128, 256], BF16, tag="t2Tsb")
            nc.vector.tensor_copy(t2T_sb[:], t2T_ps[:])
            for t in range(2):
                t2_ps = small_ps.tile([128, 128], BF16, tag="smps")
                nc.tensor.transpose(t2_ps[:], t2T_sb[:, t * 128:(t + 1) * 128],
                                    id_sb[:])
                nc.vector.tensor_copy(t2p_sb[:, t * 134 + 3:t * 134 + 131],
                                      t2_ps[:])
            t4T_sb = sb_small.tile([128, 256], BF16, tag="t4Tsb")
            for t in range(2):
                t4_ps = small_ps.tile([128, 128], F32, tag="smps")
                for ki, kh in enumerate(range(3)):
                    nc.tensor.matmul(
                        t4_ps[:], w4t_sb[:, (kh * 2 + t) * 128:(kh * 2 + t + 1) * 128],
                        t2p_sb[:, t * 134 + 3 * kh:t * 134 + 3 * kh + 128],
                        start=(ki == 0), stop=(ki == 2))
                t4_sb = sb_small.tile([128, 128], BF16, tag="t4sb")
                nc.vector.tensor_copy(t4_sb[:], t4_ps[:])
                t4T_ps = small_ps.tile([128, 128], BF16, tag="smps")
                nc.tensor.transpose(t4T_ps[:], t4_sb[:], id_sb[:])
                nc.vector.tensor_copy(t4T_sb[:, t * 128:(t + 1) * 128], t4T_ps[:])
            # t7T[g] = (t4T - t3T[g]/56)/16, with the /16 pre-folded into w4t
            t7T_sb = [sb_small.tile([128, 256], BF16, name=f"t7T{g}", tag=f"t7T{g}")
                      for g in range(2)]
            for g in range(2):
                nc.vector.scalar_tensor_tensor(t7T_sb[g][:], t3T_ps[g][:],
                                               -inv56_16,
                                               t4T_sb[:], op0=MUL, op1=ADD)
            return t7T_sb

        def emit_finals(s, t7T_sb, t9_sb):
            for j in range(NCH2):
                for ct in range(2):
                    o_ps = small_ps.tile([128, CH_OUT], F32, name="o_ps",
                                         tag="smps")
                    for g in range(2):
                        nc.tensor.matmul(
                            o_ps[:],
                            t7T_sb[g][:, ct * 128:(ct + 1) * 128],
                            t9_sb[2 * j + g][:],
                            start=(g == 0), stop=(g == 1))
                    o_sb = out_pool.tile([128, CH_OUT], BF16, tag="osb")
                    nc.scalar.copy(o_sb[:], o_ps[:])
                    nc.sync.dma_start(
                        out_dram[s, ct, :, j * CH_OUT:(j + 1) * CH_OUT], o_sb[:])

        for s in range(SPC):
            if s == 0:
                xpad_t = xpad_t0
            else:
                xpad_t = xpad_pool.tile([128, 2 * HWPM], BF16, tag="xpad")
                nc.sync.dma_start(
                    xpad_t[:].rearrange("p (t f) -> p t f", t=2),
                    d["xpad"][s].rearrange("t p f -> p t f"))
            t9_sb = emit_spatial(s, xpad_t)
            t2T_ps, t3T_ps = emit_phase1(s)
            t7T_sb = emit_chain(t2T_ps, t3T_ps)
            emit_finals(s, t7T_sb, t9_sb)


# ---------------------------------------------------------------------------
# host-side input preparation
# ---------------------------------------------------------------------------

def _prep_shared(p1, p2, w4, w5, w8):
    p1 = np.asarray(p1, np.float32)[0]          # [C,H,W]
    p2 = np.asarray(p2, np.float32)[..., 0]     # [H,W,K]
    w4 = np.asarray(w4, np.float32)
    w5 = np.asarray(w5, np.float32)
    w8 = np.asarray(w8, np.float32)

    p1t = np.zeros((HWPAD, 256), np.float32)
    p1t[:HW] = p1.reshape(C, HW).T
    p2f = np.zeros((HWPAD, 128), np.float32)
    p2f[:HW] = p2.reshape(HW, 128)

    def blockdiag_T(w, kh, kw):
        # out[t][ci, co] = w[t*128+co, ci_local, kh, kw] iff ci//4 == co//4
        out = np.zeros((2, 32, 4, 32, 4), np.float32)
        v = w.reshape(2, 32, 4, 4, 3, 3)        # [t, grp, co_l, ci_l, kh, kw]
        r = np.arange(32)
        out[:, r, :, r, :] = v[:, :, :, :, kh, kw].transpose(1, 0, 3, 2)
        return out.reshape(2, 128, 128)

    # fold the final 1/sqrt(C) output scale into w4 (t4's only consumer is
    # t7, which feeds the already-rescaled final matmul)
    w4 = w4 / np.float32(np.sqrt(np.float32(C)))
    w4t = np.stack([blockdiag_T(w4, kh, 1) for kh in range(3)])          # [3,2,...]
    w8t = np.stack([np.stack([blockdiag_T(w8, kh, kw) for kw in range(3)])
                    for kh in range(3)])                                  # [3,3,2,...]
    w5t = np.zeros((2, 2, 128, 128), np.float32)
    for dt_ in range(2):
        for cc in range(2):
            w5t[dt_, cc] = w5[dt_ * 128:(dt_ + 1) * 128,
                              cc * 128:(cc + 1) * 128].T
    ident = np.eye(128, dtype=np.float32)
    # partition-major layouts: [128, chunk-or-tap * free]
    p1t_pm = p1t.reshape(NCHUNK, 128, 256).transpose(1, 0, 2).reshape(128, -1)
    p2f_pm = p2f.reshape(NCHUNK, 128, 128).transpose(1, 0, 2).reshape(128, -1)
    w4t_pm = w4t.reshape(6, 128, 128).transpose(1, 0, 2).reshape(128, -1)
    w8t_pm = w8t.reshape(18, 128, 128).transpose(1, 0, 2).reshape(128, -1)
    w5t_pm = w5t.reshape(4, 128, 128).transpose(1, 0, 2).reshape(128, -1)
    return dict(p1t=np.ascontiguousarray(p1t_pm).astype(NPBF),
                p2f=np.ascontiguousarray(p2f_pm).astype(NPBF),
                w4t=np.ascontiguousarray(w4t_pm).astype(NPBF),
                w5t=np.ascontiguousarray(w5t_pm).astype(NPBF),
                w8t=np.ascontiguousarray(w8t_pm).astype(NPBF),
                ident=ident.astype(NPBF))


def _prep_core(x_shard):
    # x_shard: [SPC, C, H, W]
    xs = np.asarray(x_shard, np.float32)
    xpad = np.zeros((SPC, 2, 128, HWPM), NPBF)
    xpw = np.zeros((SPC, 2, 128, H, PW), NPBF)
    xpw[:, :, :, :, 3:3 + W] = xs.reshape(SPC, 2, 128, H, W).astype(NPBF)
    xpad[:, :, :, XM:XM + HWP] = xpw.reshape(SPC, 2, 128, HWP)
    xt = np.zeros((SPC, HWPAD, 256), NPBF)
    xt[:, :HW] = xs.reshape(SPC, C, HW).transpose(0, 2, 1).astype(NPBF)
    return dict(xpad=xpad,
                xt=xt.reshape(SPC, NCHUNK, 128, 256))


def kernel(x, p1, p2, w4, w5, w8):
    if "nc" not in _PROGRAM_CACHE:
        _PROGRAM_CACHE["nc"] = _build_program()
    nc = _PROGRAM_CACHE["nc"]

    shared = _prep_shared(p1, p2, w4, w5, w8)
    x = np.asarray(x, np.float32)
    in_maps = []
    for c in range(NCORE):
        m = dict(shared)
        m.update(_prep_core(x[c * SPC:(c + 1) * SPC]))
        in_maps.append(m)

    res = run_bass_kernel_spmd(nc, in_maps, core_ids=list(range(NCORE)))
    outs = []
    for c in range(NCORE):
        o = np.asarray(res.results[c]["out"], dtype=np.float32)  # [SPC,2,128,HW]
        outs.append(o.reshape(SPC, C, H, W))
    return np.concatenate(outs, axis=0)
